# revision 14
# baseline (speedup 1.0000x reference)
"""Trainium2 Bass kernel for nn_Concat_26147760898611.

Mean-pool over the word dim of article_concat [256, 2048, 300] and
options_concat [256, 64, 300], concat features -> [256, 600].

Sharding: pure data parallel over batch across 8 NeuronCores
(32 batches per core).

Per-core design (v5 — batch-pair tiles, fat 38.4 KB descriptors):
  - HWDGE descriptor dealing (measured): a DMA with n descriptors is
    dealt to k = (largest divisor of n <= 16) engines in consecutive
    chunks; only uniform 128-descriptor DMAs keep all 16 SDMA engines
    in their steady rhythm — any partial-engine DMA triggers a ~15 us
    global throughput dip (measured on v3/v4 of this kernel), so every
    data DMA here is exactly 128 descriptors.
  - Article batches are loaded in PAIRS: tile [128, 32, 300] where
    partition p holds words [32p, 32p+32) of the concatenated 2-batch
    stream — 38.4 KB contiguous per partition (double the descriptor
    size of the 1-batch layout, halving per-descriptor overhead, which
    is what limits the slowest SDMA engine).  2048 = 64*32, so
    partitions 0-63 hold batch 2t and 64-127 hold batch 2t+1 exactly.
  - Reduction per pair: DVE folds 32 -> 16 -> 8 -> 4, then 4 PE matmuls
    with a sliding TWO-hot selector (rows 0-63 -> psum row 2t, rows
    64-127 -> row 2t+1).  Selector values are 1/2048 (1/64 for options)
    so PSUM holds the mean directly and the Scalar engine (and its ACT
    table preamble load) is never used; DVE copies PSUM -> out tile.
  - Options: partition p holds 16 consecutive words of batch p//4, one
    block-selector reduction, drained into the output tile early.
  - The last pair is split into shrinking column chunks so the
    post-last-DMA tail (fold + matmul + copy + store) is short.
  - A burst of dummy matmuls at kernel start warms the PE HAM clock
    gate (1.2 -> 2.4 GHz) before real data lands.

Self-contained: hardcodes all shapes; no file reads.
"""

import numpy as np

N_CORES = 8
B = 256  # full batch
BC = B // N_CORES  # 32 batches per core
DIM = 300
AW = 2048  # article words per batch
OW = 64  # options words per batch
P = 128  # SBUF partitions
PAIRS = BC // 2  # 16 article batch-pairs per core
PW = 2 * AW // P  # 32 words per partition per pair

TAIL_CHUNKS = [16, 16]  # column split of the final pair
DATA_BUFS = 3
FOLD_BUFS = 2
WARMUP_MMS = 12

_CACHE = {}


def _build_nc():
    import concourse.bacc as bacc
    import concourse.mybir as mybir
    import concourse.tile as tile

    f32 = mybir.dt.float32
    nc = bacc.Bacc("TRN2", target_bir_lowering=False, debug=False)

    art = nc.dram_tensor("article", [BC, AW, DIM], f32, kind="ExternalInput")
    # options words (4800 f32) + sel_a row (64) + sel_o row (32) per
    # partition, packed so one fat-descriptor DMA loads all three
    optsel = nc.dram_tensor(
        "optsel", [P, OW * BC // P * DIM + 3 * BC], f32, kind="ExternalInput"
    )
    out = nc.dram_tensor("out", [BC, 2 * DIM], f32, kind="ExternalOutput")

    # pair t view [128, 32, 300]: partition p <- words 32p..32p+31 of the
    # 4096-word pair stream (partitions 0-63 = batch 2t, 64-127 = 2t+1)
    art_flat = art.ap().rearrange("b w f -> (b w) f")

    def pair_view(t):
        return art_flat[t * 2 * AW : (t + 1) * 2 * AW].rearrange(
            "(p w) f -> p w f", p=P
        )


    with tile.TileContext(nc) as tc:
        with (
            tc.tile_pool(name="const", bufs=1) as cpool,
            tc.tile_pool(name="data", bufs=DATA_BUFS) as dpool,
            tc.tile_pool(name="fold", bufs=FOLD_BUFS) as fpool,
            tc.tile_pool(name="outp", bufs=1) as opool,
            tc.tile_pool(name="psum", bufs=1, space="PSUM") as ppool,
        ):
            os_t = cpool.tile([P, OW * BC // P * DIM + 3 * BC], f32, tag="os")
            # views into the packed tile: options [128, 16, 300], selectors
            opt_v = os_t[:, 0 : 16 * DIM].rearrange("p (w f) -> p w f", f=DIM)
            sel_a_t = os_t[:, 16 * DIM : 16 * DIM + 2 * BC]
            sel_o_t = os_t[:, 16 * DIM + 2 * BC : 16 * DIM + 3 * BC]
            out_t = opool.tile([BC, 2 * DIM], f32, tag="out")

            psum_a = ppool.tile([BC, DIM], f32, tag="psum_a")
            psum_b = ppool.tile([BC, DIM], f32, tag="psum_b")
            psum_w = ppool.tile([BC, BC], f32, tag="psum_w")

            def sel_pair(t):
                # window where col BC-2 -> psum row 2t, col BC-1 -> 2t+1
                return sel_a_t[:, BC - 2 - 2 * t : 2 * BC - 2 - 2 * t]

            def reduce_tile(t, nch, sel_ap, psum, first, last, fold_k=3):
                """Fold nch cols fold_k times on DVE, matmul-reduce the rest."""
                cur, n = t, nch
                for lvl in range(fold_k):
                    if n == 1:
                        break
                    n //= 2
                    # only the matmul-facing level needs double buffering;
                    # DVE-internal levels are serialized by program order
                    nxt = fpool.tile(
                        [P, n, DIM], f32, tag=f"fold_{n}", bufs=2 if n == 4 else 1
                    )
                    nc.vector.tensor_add(nxt[:], cur[:, 0:n, :], cur[:, n : 2 * n, :])
                    cur = nxt
                for j in range(n):
                    nc.tensor.matmul(
                        psum[:], sel_ap, cur[:, j, :],
                        start=(first and j == 0), stop=(last and j == n - 1),
                    )

            # first pair's DMA leads the queue
            t0 = dpool.tile([P, PW, DIM], f32, tag="data")
            nc.sync.dma_start(t0[:], pair_view(0))
            nc.sync.dma_start(os_t[:], optsel.ap()[:])

            # PE warmup: flip the HAM clock gate to 2.4 GHz early.
            for _ in range(WARMUP_MMS):
                nc.tensor.matmul(
                    psum_w[:], sel_o_t[:], sel_a_t[:, 0:BC], start=True, stop=True
                )

            reduce_tile(t0, PW, sel_pair(0), psum_a, True, False)

            # options; drain its psum into the output tile early
            reduce_tile(opt_v, 16, sel_o_t[:], psum_b, True, True,
                        fold_k=2)
            nc.vector.tensor_copy(out_t[:, DIM : 2 * DIM], psum_b[:])
            nc.sync.dma_start(out.ap()[:, DIM : 2 * DIM], out_t[:, DIM : 2 * DIM])

            for t in range(1, PAIRS - 1):
                tl = dpool.tile([P, PW, DIM], f32, tag="data")
                nc.sync.dma_start(tl[:], pair_view(t))
                reduce_tile(tl, PW, sel_pair(t), psum_a, False, False)

            # final pair in shrinking column chunks -> the very last DMA is
            # small and its fold+matmul tail is short
            tp = PAIRS - 1
            pv = pair_view(tp)
            sel_last = sel_pair(tp)
            assert sum(TAIL_CHUNKS) == PW
            c0 = 0
            for i, nch in enumerate(TAIL_CHUNKS):
                tl = dpool.tile([P, nch, DIM], f32, tag="data")
                nc.sync.dma_start(tl[:], pv[:, c0 : c0 + nch, :])
                reduce_tile(
                    tl, nch, sel_last, psum_a, False, i == len(TAIL_CHUNKS) - 1,
                    fold_k=2,
                )
                c0 += nch

            nc.vector.tensor_copy(out_t[:, 0:DIM], psum_a[:])
            nc.sync.dma_start(out.ap()[:, 0:DIM], out_t[:, 0:DIM])

    nc.compile()
    return nc


def get_nc():
    if "nc" not in _CACHE:
        _CACHE["nc"] = _build_nc()
    return _CACHE["nc"]


def _sel_arrays():
    # selector values carry the mean scaling (exact powers of two).
    # sel_a: two-hot sliding selector for batch pairs — window
    # [BC-2-2t, 2*BC-2-2t) puts col BC-2 at psum row 2t (partitions 0-63)
    # and col BC-1 at row 2t+1 (partitions 64-127).
    sel_a = np.zeros((P, 2 * BC), np.float32)
    sel_a[0:64, BC - 2] = 1.0 / AW
    sel_a[64:P, BC - 1] = 1.0 / AW
    sel_o = np.zeros((P, BC), np.float32)
    sel_o[np.arange(P), np.arange(P) // (P // BC)] = 1.0 / OW
    return sel_a, sel_o


def make_in_maps(article, options):
    article = np.ascontiguousarray(np.asarray(article, dtype=np.float32))
    options = np.ascontiguousarray(np.asarray(options, dtype=np.float32))
    assert article.shape == (B, AW, DIM), article.shape
    assert options.shape == (B, OW, DIM), options.shape
    sel_a, sel_o = _sel_arrays()
    maps = []
    for i in range(N_CORES):
        opt_core = options[i * BC : (i + 1) * BC].reshape(P, OW * BC // P * DIM)
        optsel = np.ascontiguousarray(
            np.concatenate([opt_core, sel_a, sel_o], axis=1)
        )
        maps.append(
            {"article": article[i * BC : (i + 1) * BC], "optsel": optsel}
        )
    return maps


def run_sharded(article, options, **spmd_kwargs):
    from concourse.bass_utils import run_bass_kernel_spmd

    nc = get_nc()
    in_maps = make_in_maps(article, options)
    res = run_bass_kernel_spmd(nc, in_maps, list(range(N_CORES)), **spmd_kwargs)
    full = np.concatenate(
        [res.results[i]["out"] for i in range(N_CORES)], axis=0
    ).astype(np.float32)
    return full, res


def kernel(article_concat, options_concat):
    full, _ = run_sharded(article_concat, options_concat)
    return full


# revision 15
# speedup vs baseline: 1.0095x; 1.0095x over previous
"""Trainium2 Bass kernel for nn_Concat_26147760898611.

Mean-pool over the word dim of article_concat [256, 2048, 300] and
options_concat [256, 64, 300], concat features -> [256, 600].

Sharding: pure data parallel over batch across 8 NeuronCores
(32 batches per core).

Per-core design (v5 — batch-pair tiles, fat 38.4 KB descriptors):
  - HWDGE descriptor dealing (measured): a DMA with n descriptors is
    dealt to k = (largest divisor of n <= 16) engines in consecutive
    chunks; only uniform 128-descriptor DMAs keep all 16 SDMA engines
    in their steady rhythm — any partial-engine DMA triggers a ~15 us
    global throughput dip (measured on v3/v4 of this kernel), so every
    data DMA here is exactly 128 descriptors.
  - Article batches are loaded in PAIRS: tile [128, 32, 300] where
    partition p holds words [32p, 32p+32) of the concatenated 2-batch
    stream — 38.4 KB contiguous per partition (double the descriptor
    size of the 1-batch layout, halving per-descriptor overhead, which
    is what limits the slowest SDMA engine).  2048 = 64*32, so
    partitions 0-63 hold batch 2t and 64-127 hold batch 2t+1 exactly.
  - Reduction per pair: DVE folds 32 -> 16 -> 8 -> 4, then 4 PE matmuls
    with a sliding TWO-hot selector (rows 0-63 -> psum row 2t, rows
    64-127 -> row 2t+1).  Selector values are 1/2048 (1/64 for options)
    so PSUM holds the mean directly and the Scalar engine (and its ACT
    table preamble load) is never used; DVE copies PSUM -> out tile.
  - Options: partition p holds 16 consecutive words of batch p//4, one
    block-selector reduction, drained into the output tile early.
  - The last pair is split into shrinking column chunks so the
    post-last-DMA tail (fold + matmul + copy + store) is short.
  - A burst of dummy matmuls at kernel start warms the PE HAM clock
    gate (1.2 -> 2.4 GHz) before real data lands.

Self-contained: hardcodes all shapes; no file reads.
"""

import numpy as np

N_CORES = 8
B = 256  # full batch
BC = B // N_CORES  # 32 batches per core
DIM = 300
AW = 2048  # article words per batch
OW = 64  # options words per batch
P = 128  # SBUF partitions
PAIRS = BC // 2  # 16 article batch-pairs per core
PW = 2 * AW // P  # 32 words per partition per pair

TAIL_CHUNKS = [16, 16]  # column split of the final pair
DATA_BUFS = 3
FOLD_BUFS = 2
WARMUP_MMS = 12

_CACHE = {}


def _build_nc():
    import concourse.bacc as bacc
    import concourse.mybir as mybir
    import concourse.tile as tile

    f32 = mybir.dt.float32
    nc = bacc.Bacc("TRN2", target_bir_lowering=False, debug=False)

    art = nc.dram_tensor("article", [BC, AW, DIM], f32, kind="ExternalInput")
    # options words (4800 f32) + sel_a row (64) + sel_o row (32) per
    # partition, packed so one fat-descriptor DMA loads all three
    optsel = nc.dram_tensor(
        "optsel", [P, OW * BC // P * DIM + 3 * BC], f32, kind="ExternalInput"
    )
    out = nc.dram_tensor("out", [BC, 2 * DIM], f32, kind="ExternalOutput")

    # pair t view [128, 32, 300]: partition p <- words 32p..32p+31 of the
    # 4096-word pair stream (partitions 0-63 = batch 2t, 64-127 = 2t+1)
    art_flat = art.ap().rearrange("b w f -> (b w) f")

    def pair_view(t):
        return art_flat[t * 2 * AW : (t + 1) * 2 * AW].rearrange(
            "(p w) f -> p w f", p=P
        )


    with tile.TileContext(nc) as tc:
        with (
            tc.tile_pool(name="const", bufs=1) as cpool,
            tc.tile_pool(name="data", bufs=DATA_BUFS) as dpool,
            tc.tile_pool(name="fold", bufs=FOLD_BUFS) as fpool,
            tc.tile_pool(name="outp", bufs=1) as opool,
            tc.tile_pool(name="psum", bufs=1, space="PSUM") as ppool,
        ):
            os_t = cpool.tile([P, OW * BC // P * DIM + 3 * BC], f32, tag="os")
            # views into the packed tile: options [128, 16, 300], selectors
            opt_v = os_t[:, 0 : 16 * DIM].rearrange("p (w f) -> p w f", f=DIM)
            sel_a_t = os_t[:, 16 * DIM : 16 * DIM + 2 * BC]
            sel_o_t = os_t[:, 16 * DIM + 2 * BC : 16 * DIM + 3 * BC]
            out_t = opool.tile([BC, 2 * DIM], f32, tag="out")

            psum_a = ppool.tile([BC, DIM], f32, tag="psum_a")
            psum_b = ppool.tile([BC, DIM], f32, tag="psum_b")
            psum_w = ppool.tile([BC, BC], f32, tag="psum_w")

            def sel_pair(t):
                # window where col BC-2 -> psum row 2t, col BC-1 -> 2t+1
                return sel_a_t[:, BC - 2 - 2 * t : 2 * BC - 2 - 2 * t]

            def reduce_tile(t, nch, sel_ap, psum, first, last, fold_k=3):
                """Fold nch cols fold_k times on DVE, matmul-reduce the rest."""
                cur, n = t, nch
                for lvl in range(fold_k):
                    if n == 1:
                        break
                    n //= 2
                    nxt = fpool.tile([P, n, DIM], f32, tag=f"fold_{n}")
                    nc.vector.tensor_add(nxt[:], cur[:, 0:n, :], cur[:, n : 2 * n, :])
                    cur = nxt
                for j in range(n):
                    nc.tensor.matmul(
                        psum[:], sel_ap, cur[:, j, :],
                        start=(first and j == 0), stop=(last and j == n - 1),
                    )

            # first pair's DMA leads the queue
            t0 = dpool.tile([P, PW, DIM], f32, tag="data")
            nc.sync.dma_start(t0[:], pair_view(0))
            nc.sync.dma_start(os_t[:], optsel.ap()[:])

            # PE warmup: flip the HAM clock gate to 2.4 GHz early.
            for _ in range(WARMUP_MMS):
                nc.tensor.matmul(
                    psum_w[:], sel_o_t[:], sel_a_t[:, 0:BC], start=True, stop=True
                )

            reduce_tile(t0, PW, sel_pair(0), psum_a, True, False)

            # options; drain its psum into the output tile early
            reduce_tile(opt_v, 16, sel_o_t[:], psum_b, True, True,
                        fold_k=2)
            nc.vector.tensor_copy(out_t[:, DIM : 2 * DIM], psum_b[:])
            nc.sync.dma_start(out.ap()[:, DIM : 2 * DIM], out_t[:, DIM : 2 * DIM])

            for t in range(1, PAIRS - 1):
                tl = dpool.tile([P, PW, DIM], f32, tag="data")
                nc.sync.dma_start(tl[:], pair_view(t))
                reduce_tile(tl, PW, sel_pair(t), psum_a, False, False)

            # final pair in shrinking column chunks -> the very last DMA is
            # small and its fold+matmul tail is short
            tp = PAIRS - 1
            pv = pair_view(tp)
            sel_last = sel_pair(tp)
            assert sum(TAIL_CHUNKS) == PW
            c0 = 0
            for i, nch in enumerate(TAIL_CHUNKS):
                tl = dpool.tile([P, nch, DIM], f32, tag="data")
                nc.sync.dma_start(tl[:], pv[:, c0 : c0 + nch, :])
                reduce_tile(
                    tl, nch, sel_last, psum_a, False, i == len(TAIL_CHUNKS) - 1,
                    fold_k=2,
                )
                c0 += nch

            nc.vector.tensor_copy(out_t[:, 0:DIM], psum_a[:])
            nc.sync.dma_start(out.ap()[:, 0:DIM], out_t[:, 0:DIM])

    nc.compile()
    return nc


def get_nc():
    if "nc" not in _CACHE:
        _CACHE["nc"] = _build_nc()
    return _CACHE["nc"]


def _sel_arrays():
    # selector values carry the mean scaling (exact powers of two).
    # sel_a: two-hot sliding selector for batch pairs — window
    # [BC-2-2t, 2*BC-2-2t) puts col BC-2 at psum row 2t (partitions 0-63)
    # and col BC-1 at row 2t+1 (partitions 64-127).
    sel_a = np.zeros((P, 2 * BC), np.float32)
    sel_a[0:64, BC - 2] = 1.0 / AW
    sel_a[64:P, BC - 1] = 1.0 / AW
    sel_o = np.zeros((P, BC), np.float32)
    sel_o[np.arange(P), np.arange(P) // (P // BC)] = 1.0 / OW
    return sel_a, sel_o


def make_in_maps(article, options):
    article = np.ascontiguousarray(np.asarray(article, dtype=np.float32))
    options = np.ascontiguousarray(np.asarray(options, dtype=np.float32))
    assert article.shape == (B, AW, DIM), article.shape
    assert options.shape == (B, OW, DIM), options.shape
    sel_a, sel_o = _sel_arrays()
    maps = []
    for i in range(N_CORES):
        opt_core = options[i * BC : (i + 1) * BC].reshape(P, OW * BC // P * DIM)
        optsel = np.ascontiguousarray(
            np.concatenate([opt_core, sel_a, sel_o], axis=1)
        )
        maps.append(
            {"article": article[i * BC : (i + 1) * BC], "optsel": optsel}
        )
    return maps


def run_sharded(article, options, **spmd_kwargs):
    from concourse.bass_utils import run_bass_kernel_spmd

    nc = get_nc()
    in_maps = make_in_maps(article, options)
    res = run_bass_kernel_spmd(nc, in_maps, list(range(N_CORES)), **spmd_kwargs)
    full = np.concatenate(
        [res.results[i]["out"] for i in range(N_CORES)], axis=0
    ).astype(np.float32)
    return full, res


def kernel(article_concat, options_concat):
    full, _ = run_sharded(article_concat, options_concat)
    return full
